# revision 38
# baseline (speedup 1.0000x reference)
"""Trainium2 Bass kernel for nn_AdvectionDiffusionReaction2M (v2).

Advection-diffusion-reaction on a 512x512 grid, 199 sequential steps, output =
all intermediate states (199,512,512) f32.

Sharding: rows split 8 ways (64 rows/core) with 16-row ghost zones refreshed
by an AllGather every 16 steps.  SBUF layout per core: flat [128, 576] f32,
viewed as [p, b, i]: partition p = column group (cols 4p..4p+3 at blocks
b=1..4), b=0/5 = ghost columns 4p-1 / 4p+4 (refreshed each step by a
partition-shift matmul on PE), i = stored row (96 = 16 ghost + 64 + 16 ghost).

v2: the update is regrouped per neighbor with Tc-dependent coefficients
   Tn = Up*(s+h*Tc^2) + Dn*(s-h*Tc^2) + L*(s-h*Tc) + R*(s+h*Tc) + phi(Tc)
   phi = Tc + g*(Tc^3-Tc^2+Tc),  g = h*2dx
computed by 4 fused custom DVE ops over contiguous [128,384] views (block-edge
rows are sacrificial ghost rows, so row-crossing garbage is harmless).
"""

import os
import numpy as np

N = 512
DX = 1.0 / (N - 1)
DT = 1e-7
MB = 256
NCORES = 8
K = 16                      # ghost depth (rows)
RS = 64 + 2 * K             # stored rows per core (96)
NSTEPS = int(os.environ.get("ADR_NSTEPS", "199"))
B = RS                      # block stride in flat free dim
CAD = 12                    # sync cadence (K - CAD = slack for async collective)
LAG = 2                     # ghost integration happens LAG steps after trigger

LAST_EXEC_NS = None
LAST_RESULT = None

_OPS_REGISTERED = {}


def _register_ops():
    """Register custom DVE ops (runtime registration into dve_ops.OPS)."""
    if _OPS_REGISTERED:
        return _OPS_REGISTERED
    import concourse.dve_ops as dve_ops
    from concourse.dve_ops import DveOp, OPS
    from concourse.dve_spec import Spec, Src0, Src1, C0, C1, C2, One, sq, lower
    from concourse.dve_uop import DveOpSpec

    def make_op(name, body, reference):
        for op in OPS:
            if op.name == name:
                return op
        spec = Spec(body=body, reference=reference)
        shas = {}
        for ver in ("v3", "v4"):
            uops = lower(spec, ver=ver)
            tmp = DveOpSpec(name=name, opcode=0, uops=uops, rd1_en=True)
            shas[ver] = tmp.sha(ver)
        op = DveOp(name, spec, subdim=False, uops_sha=shas)
        OPS.append(op)
        dve_ops._SUB_OPCODE_FOR_NAME[name] = (
            dve_ops._CUSTOM_DVE_ROW_BASE + len(OPS) - 1)
        assert dve_ops._SUB_OPCODE_FOR_NAME[name] < 0x20, "opcode row overflow"
        dve_ops.CUSTOM_DVE_SPECS[name] = spec
        return op

    q = sq(Src0)
    gc = C0 * C2                          # g = h * 2dx (hoisted mult)
    # out = Up*(s + h*Tc^2) + g*(Tc^2 - Tc)*Tc      [phi part 1: g(Tc^3-Tc^2)]
    _OPS_REGISTERED["APHI"] = make_op(
        "ADR_APHI",
        Src1 * (C1 + q * C0) + (q - Src0) * gc * Src0,
        lambda in0, in1, s0, s1, imm2:
            in1 * (s1 + in0**2 * s0)
            + (in0**2 - in0) * (s0 * imm2) * in0)
    # out = Dn*(s - h*Tc^2)
    _OPS_REGISTERED["BSQ"] = make_op(
        "ADR_BSQ", Src1 * (C1 - q * C0),
        lambda in0, in1, s0, s1: in1 * (s1 - in0**2 * s0))
    # out = L*(s - h*Tc) + (h*Tc)*2dx               [phi part 3: g*Tc]
    _a = Src0 * C0
    _OPS_REGISTERED["CLIN"] = make_op(
        "ADR_CLIN", Src1 * (C1 - _a) + _a * C2,
        lambda in0, in1, s0, s1, imm2:
            in1 * (s1 - in0 * s0) + in0 * s0 * imm2)
    # out = R*(s + h*Tc) + Tc + (-4)*s*Tc           [phi part 2: (1-4s)Tc]
    _OPS_REGISTERED["DLIN"] = make_op(
        "ADR_DLIN", Src1 * (C1 + _a) + Src0 + Src0 * C1 * C2,
        lambda in0, in1, s0, s1, imm2:
            in1 * (s1 + in0 * s0) + in0 + in0 * s1 * imm2)
    # out = Src0*C0 + Src1*C1  (masked blend / select)
    _OPS_REGISTERED["SEL"] = make_op(
        "ADR_SEL", Src0 * C0 + Src1 * C1,
        lambda in0, in1, s0, s1: in0 * s0 + in1 * s1)
    return _OPS_REGISTERED


def _pack_core(G, c):
    """Full grid (512,512) -> per-core tile [128, 6, RS] (f32, zero padded)."""
    lo = 64 * c - K
    S = np.zeros((RS, N), np.float32)
    g0, g1 = max(lo, 0), min(lo + RS, N)
    S[g0 - lo: g1 - lo] = G[g0:g1]
    cols = (4 * np.arange(128)[:, None] - 1 + np.arange(6)[None, :])  # [128,6]
    valid = (cols >= 0) & (cols < N)
    t = S.T[np.clip(cols, 0, N - 1)]          # [128, 6, RS]
    t[~valid] = 0.0
    return np.ascontiguousarray(t, dtype=np.float32)


def _build(nc, tile, mybir, bass, scal):
    f32 = mybir.dt.float32
    u32 = mybir.dt.uint32
    AF = mybir.ActivationFunctionType
    OP = mybir.AluOpType
    ops = _register_ops()
    APHI, BSQ, CLIN, DLIN, SEL = (ops[k] for k in
                                  ("APHI", "BSQ", "CLIN", "DLIN", "SEL"))

    bf16 = mybir.dt.bfloat16
    BF = bf16 if os.environ.get("ADR_BF16", "0") == "1" else f32
    u0s_d = nc.dram_tensor("u0s", [128, 6, RS], f32, kind="ExternalInput").ap()
    ppc_d = nc.dram_tensor("ppc", [128, 14], f32, kind="ExternalInput").ap()
    wr_d = nc.dram_tensor("wr", [128, 128], f32, kind="ExternalInput").ap()
    wl_d = nc.dram_tensor("wl", [128, 128], f32, kind="ExternalInput").ap()
    nbrs_d = nc.dram_tensor("nbrs", [1, 2], u32, kind="ExternalInput").ap()
    rsel_d = nc.dram_tensor("rsel", [1, 2], u32, kind="ExternalInput").ap()
    out_d = nc.dram_tensor("out", [NSTEPS, 128, 4, 64], f32,
                           kind="ExternalOutput").ap()

    # ghost sync every K steps (synchronous: state-t bands must merge into the
    # state-t tile before step t+1 — any lag breaks time-consistency)
    nsync = [t for t in range(K, NSTEPS, K)]

    with tile.TileContext(nc) as tc:
        with tc.tile_pool(name="state", bufs=1) as sp, \
             tc.tile_pool(name="tmp", bufs=2) as tp, \
             tc.tile_pool(name="psum", bufs=2, space="PSUM") as pp, \
             tc.tile_pool(name="dram", bufs=1, space="DRAM") as dp:

            tA = sp.tile([128, 6 * B], f32, tag="tA")
            tB = sp.tile([128, 6 * B], f32, tag="tB")
            ppc = sp.tile([128, 14], f32, tag="ppc")
            wr = sp.tile([128, 128], f32, tag="wr")
            wl = sp.tile([128, 128], f32, tag="wl")

            cc_in = dp.tile([2, 128, 6, K], f32, tag="ccin")
            cc_outs = {t: dp.tile([16 * 128, 6, K], f32, tag=f"ccout{t}",
                                  name=f"ccout{t}", addr_space="Shared")
                       for t in nsync}
            stgP = sp.tile([128, 6, K], f32, tag="stgP")
            stgN = sp.tile([128, 6, K], f32, tag="stgN")

            nc.sync.dma_start(tA[:].rearrange("p (b i) -> p b i", b=6), u0s_d[:])
            nc.sync.dma_start(ppc[:], ppc_d[:])
            nc.sync.dma_start(wr[:], wr_d[:])
            nc.sync.dma_start(wl[:], wl_d[:])

            rp = nc.alloc_registers("rprev")
            nc.regs_load(rp, nbrs_d[0:1, 0:1])
            sv_prev = nc.snap(rp, min_val=0, max_val=15 * 128)
            rn = nc.alloc_registers("rnext")
            nc.regs_load(rn, nbrs_d[0:1, 1:2])
            sv_next = nc.snap(rn, min_val=0, max_val=15 * 128)
            # per-core Neumann source rows (core 0: 17 else 16; core 7: 78 else 79)
            rt0 = nc.alloc_registers("rtop")
            nc.regs_load(rt0, rsel_d[0:1, 0:1])
            sv_rtop = nc.snap(rt0, min_val=K, max_val=K + 1)
            rb0 = nc.alloc_registers("rbot")
            nc.regs_load(rb0, rsel_d[0:1, 1:2])
            sv_rbot = nc.snap(rb0, min_val=K + 62, max_val=K + 63)

            s_ = ppc[:, 3:4]; h_ = ppc[:, 4:5]
            m0 = ppc[:, 5:6]; n0 = ppc[:, 6:7]
            m7 = ppc[:, 7:8]; n7 = ppc[:, 8:9]
            ifA = ppc[32:64, 9:10]; ifB = ppc[32:64, 10:11]; ifC = ppc[32:64, 11:12]
            cD = ppc[96:128, 12:13]; cE = ppc[96:128, 13:14]
            sq_ = lambda a, b: slice(a, b)  # noqa: E731

            cur, nxt = tA, tB
            for t in range(1, NSTEPS + 1):
                # contiguous flat views: owned blocks 1..4, all 96 rows
                Tc = cur[:, B:5 * B]
                Up = cur[:, B - 1:5 * B - 1]
                Dn = cur[:, B + 1:5 * B + 1]
                L = cur[:, 0:4 * B]
                R = cur[:, 2 * B:6 * B]
                TnV = nxt[:, B:5 * B]

                PU = tp.tile([128, 4 * B], BF, tag="PU")
                PD = tp.tile([128, 4 * B], BF, tag="PD")
                PL = tp.tile([128, 4 * B], BF, tag="PL")
                PR = tp.tile([128, 4 * B], f32, tag="PR")
                S1 = tp.tile([128, 4 * B], BF, tag="S1")
                S2 = tp.tile([128, 4 * B], BF, tag="S2")
                I1 = tp.tile([128, B], f32, tag="I1")

                # neighbor-grouped fused passes (DVE) + bf16 sum tree on DVE;
                # only DLIN's output carries state-scale values (f32).
                # APHI/BSQ read no ghost columns -> they start while the
                # previous step's ghost refresh is still in flight.
                nc.vector._custom_dve(APHI, out=PU[:], in0=Tc, in1=Up,
                                      s0=h_, s1=s_, imm2=2.0 * DX)
                nc.vector._custom_dve(BSQ, out=PD[:], in0=Tc, in1=Dn,
                                      s0=h_, s1=s_)
                nc.vector._custom_dve(CLIN, out=PL[:], in0=Tc, in1=L,
                                      s0=h_, s1=s_, imm2=2.0 * DX)
                nc.vector.tensor_tensor(S1[:], PU[:], PD[:], OP.add)
                nc.vector._custom_dve(DLIN, out=PR[:], in0=Tc, in1=R,
                                      s0=h_, s1=s_, imm2=-4.0)
                nc.vector.tensor_tensor(S2[:], S1[:], PL[:], OP.add)
                # interface precompute (reads OLD state only).  NOTE: custom
                # scalar APs only work at partition base 0 -> stock TS/STT.
                nc.vector.tensor_scalar(
                    I1[32:64, :], cur[32:64, 5 * B:6 * B], ifA, None, OP.mult)
                nc.vector.scalar_tensor_tensor(
                    I1[32:64, :], cur[32:64, 3 * B:4 * B], ifB, I1[32:64, :],
                    OP.mult, OP.add)
                nc.vector.tensor_tensor(TnV, S2[:], PR[:], OP.add)

                # interface blend into b4 [32:64) (stock STT; in-place safe)
                nc.vector.scalar_tensor_tensor(
                    nxt[32:64, 4 * B:5 * B], nxt[32:64, 4 * B:5 * B], ifC,
                    I1[32:64, :], OP.mult, OP.add)

                # row boundary (Neumann): ACT copy with per-core dynamic
                # source row (middle cores self-copy, a no-op)
                nx3 = nxt[:].rearrange("p (b i) -> p b i", b=6)
                nc.scalar.copy(nx3[:, 1:5, K:K + 1],
                               nx3[:, 1:5, bass.ds(sv_rtop, 1)])
                nc.scalar.copy(nx3[:, 1:5, K + 63:K + 64],
                               nx3[:, 1:5, bass.ds(sv_rbot, 1)])

                # column boundary: col 0 (p0) copy on ACT; col 511 (p127) blend
                nc.scalar.copy(nxt[0:1, B:2 * B], nxt[0:1, 2 * B:3 * B])
                CT = tp.tile([128, B], f32, tag="CT")
                nc.vector.tensor_scalar(
                    CT[96:128, :], nxt[96:128, 3 * B:4 * B], cD, None, OP.mult)
                nc.vector.scalar_tensor_tensor(
                    nxt[96:128, 4 * B:5 * B], nxt[96:128, 4 * B:5 * B], cE,
                    CT[96:128, :], OP.mult, OP.add)

                # ghost column refresh via partition-shift matmuls (full rows)
                psR = pp.tile([128, B], f32, tag="psR")
                psL = pp.tile([128, B], f32, tag="psL")
                nc.tensor.matmul(psR[:], wr[:], nxt[:, B:2 * B],
                                 start=True, stop=True)
                nc.tensor.matmul(psL[:], wl[:], nxt[:, 4 * B:5 * B],
                                 start=True, stop=True)
                nc.scalar.copy(nxt[:, 5 * B:6 * B], psR[:])
                nc.scalar.copy(nxt[:, 0:B], psL[:])

                # output: owned rows
                nc.sync.dma_start(out_d[t - 1], nx3[:, 1:5, K:K + 64])

                # ghost row sync (blocking; gpsimd queue keeps Sync free)
                if t in nsync:
                    cc_out = cc_outs[t]
                    nc.gpsimd.dma_start(cc_in[0], nx3[:, :, K:2 * K])
                    nc.gpsimd.dma_start(cc_in[1], nx3[:, :, 64:64 + K])
                    nc.gpsimd.collective_compute(
                        "AllGather", OP.bypass,
                        replica_groups=[list(range(NCORES))],
                        ins=[cc_in[:]], outs=[cc_out[:]])
                    nc.gpsimd.dma_start(nx3[:, :, 0:K],
                                        cc_out[bass.ds(sv_prev, 128)])
                    nc.gpsimd.dma_start(nx3[:, :, 64 + K:64 + 2 * K],
                                        cc_out[bass.ds(sv_next, 128)])

                cur, nxt = nxt, cur
    return nc


def _ensure_ntff_hook():
    """Provide antenv.axon_hooks (missing in this image) so bass_utils can
    NTFF-profile under axon."""
    import sys
    import types
    try:
        from antenv.axon_hooks import get_axon_ntff_profile_hook  # noqa: F401
        return
    except ImportError:
        pass
    mod = types.ModuleType("antenv.axon_hooks")
    mod._hook = None

    def set_axon_ntff_profile_hook(h):
        mod._hook = h

    def get_axon_ntff_profile_hook():
        return mod._hook

    mod.set_axon_ntff_profile_hook = set_axon_ntff_profile_hook
    mod.get_axon_ntff_profile_hook = get_axon_ntff_profile_hook
    sys.modules["antenv.axon_hooks"] = mod
    import antenv
    antenv.axon_hooks = mod
    try:
        from trn_agent_boot.trn_boot import _ntff_profile_via_ctypes
        hook = _ntff_profile_via_ctypes("/opt/axon/libaxon_pjrt.so")
        if hook is not None:
            mod._hook = hook
    except Exception:
        pass


def kernel(u0, k1, k2, alpha1, alpha2):
    global LAST_EXEC_NS, LAST_RESULT
    import concourse.bacc as bacc
    import concourse.bass as bass
    import concourse.tile as tile
    import concourse.mybir as mybir
    from concourse.bass_utils import run_bass_kernel_spmd

    u0 = np.asarray(u0, dtype=np.float32)
    k1f = float(np.asarray(k1).reshape(-1)[0])
    k2f = float(np.asarray(k2).reshape(-1)[0])
    a1f = float(np.asarray(alpha1).reshape(-1)[0])
    a2f = float(np.asarray(alpha2).reshape(-1)[0])

    dx2 = DX * DX
    scal = {"ca": k1f / (k1f + k2f), "cb": k2f / (k1f + k2f)}

    nc = bacc.Bacc(
        "TRN2", target_bir_lowering=False, debug=False,
        num_devices=NCORES,
    )
    _build(nc, tile, mybir, bass, scal)
    nc.compile()

    left = np.arange(128) < 64
    s = np.where(left, DT * a1f / dx2, DT * a2f / dx2).astype(np.float32)
    h = np.where(left, DT * k1f / (2 * DX), DT * k2f / (2 * DX)).astype(np.float32)
    g = np.where(left, DT * k1f, DT * k2f).astype(np.float32)
    WR = np.eye(128, k=-1, dtype=np.float32)   # out[m] = in[m+1]
    WL = np.eye(128, k=+1, dtype=np.float32)   # out[m] = in[m-1]

    m63 = (np.arange(128) == 63).astype(np.float32)
    m127 = (np.arange(128) == 127).astype(np.float32)
    in_maps = []
    for c in range(NCORES):
        ppc = np.zeros((128, 14), np.float32)
        ppc[:, 0] = g
        ppc[:, 1] = -g
        ppc[:, 2] = 1.0 - 4.0 * s + g
        ppc[:, 3] = s
        ppc[:, 4] = h
        ppc[:, 5] = 1.0 if c == 0 else 0.0
        ppc[:, 6] = 0.0 if c == 0 else 1.0
        ppc[:, 7] = 1.0 if c == NCORES - 1 else 0.0
        ppc[:, 8] = 0.0 if c == NCORES - 1 else 1.0
        ppc[:, 9] = m63 * scal["ca"]       # ifA
        ppc[:, 10] = m63 * scal["cb"]      # ifB
        ppc[:, 11] = 1.0 - m63             # ifC
        ppc[:, 12] = m127                  # cD
        ppc[:, 13] = 1.0 - m127            # cE
        prev_off = (2 * (c - 1) + 1) * 128 if c > 0 else 0
        next_off = (2 * (c + 1)) * 128 if c < NCORES - 1 else 0
        rtop = K + 1 if c == 0 else K
        rbot = K + 62 if c == NCORES - 1 else K + 63
        in_maps.append({
            "u0s": _pack_core(u0, c),
            "ppc": ppc,
            "wr": WR,
            "wl": WL,
            "nbrs": np.array([[prev_off, next_off]], dtype=np.uint32),
            "rsel": np.array([[rtop, rbot]], dtype=np.uint32),
        })

    trace = os.environ.get("ADR_TRACE", "0") == "1"
    if trace:
        _ensure_ntff_hook()
    res = run_bass_kernel_spmd(
        nc, in_maps, core_ids=list(range(NCORES)), trace=trace)
    LAST_EXEC_NS = res.exec_time_ns
    LAST_RESULT = res

    full = np.zeros((NSTEPS, N, N), np.float32)
    for c in range(NCORES):
        arr = np.asarray(res.results[c]["out"]).reshape(NSTEPS, 128, 4, 64)
        full[:, 64 * c:64 * (c + 1), :] = (
            arr.transpose(0, 3, 1, 2).reshape(NSTEPS, 64, 512))
    return full
